# revision 20
# baseline (speedup 1.0000x reference)
"""Trainium2 Bass kernel: bidirectional GNN message passing (scatter-add) + concat.

Computation (per batch b):
    out[b, :, 0:256]   = M_b @ x[b]        where M_b[i, j] = (# edges i<-j) + (# edges j<-i)
    out[b, :, 256:512] = x[b]

M_b is a symmetric count matrix built on the host from the edge indices (pure
index preprocessing; all x-dependent arithmetic runs on the NeuronCores).
Sharding: data-parallel over the batch dim, 4 batches per core on 8 cores.
On-device the scatter-add is computed as dense 128x128-block matmuls on the
tensor engine. The counts are stored in HBM directly as fp8e4m3 bytes (exact
for counts <= 16) so the PE consumes them with zero on-chip casting; x is the
f16 moving operand. Output rows are assembled as [scatter | x] in SBUF so the
store is one stream of 2KB-contiguous descriptors.
"""

import numpy as np

B, N, D = 32, 2048, 256
NC = 8                  # cores
BPC = B // NC           # batches per core = 4
NB = N // 128           # node blocks per batch = 16
G = BPC * NB            # node blocks per core = 64
AMERGE = 2              # dst strips per A DMA
OMERGE = 2              # strips per out DMA
NWARM = 32              # PE warmup matmuls (clock-gate ramp during initial DMA)

_compiled = None


def _build_bass():
    from contextlib import ExitStack
    import concourse.bass as bass
    import concourse.tile as tile
    from concourse import bacc, mybir

    nc = bacc.Bacc("TRN2", target_bir_lowering=False, debug=False, num_devices=NC)
    x_ap = nc.dram_tensor("x", [BPC * N, D], mybir.dt.float32, kind="ExternalInput").ap()
    # A layout [b, im, s, ii, J, d] fp8e4: each im-group of AMERGE dst-strips is a
    # flat [128, AMERGE*NB*128] block -> 4KB-contiguous DMA descriptor runs.
    a_ap = nc.dram_tensor(
        "a", [BPC, NB // AMERGE, 128, AMERGE * NB * 128], mybir.dt.float8e4, kind="ExternalInput"
    ).ap()
    out_ap = nc.dram_tensor("out", [BPC * N, 2 * D], mybir.dt.float32, kind="ExternalOutput").ap()

    with tile.TileContext(nc) as tc:
        with ExitStack() as ctx:
            xpool = ctx.enter_context(tc.tile_pool(name="x", bufs=1))
            xhpool = ctx.enter_context(tc.tile_pool(name="xh", bufs=1))
            # A prefetch of 6 strips: enough runway that a4/a5 issue before
            # tile 0 is consumed (they otherwise land ~8us late and stall
            # tiles 7-8), while the ramp-phase x still gets its share via the
            # sync-ring interleave below.
            apool = ctx.enter_context(tc.tile_pool(name="a8", bufs=6))
            pspool = ctx.enter_context(tc.tile_pool(name="ps", bufs=6, space="PSUM"))
            wpool = ctx.enter_context(tc.tile_pool(name="wm", bufs=1))
            wpspool = ctx.enter_context(tc.tile_pool(name="wps", bufs=1, space="PSUM"))
            opool = ctx.enter_context(tc.tile_pool(name="o", bufs=6))

            # x resident in SBUF: [p, (g, d)] where node n = g*128 + p.
            x_sb = xpool.tile([128, G * D], mybir.dt.float32)
            x_h = xhpool.tile([128, G * D], mybir.dt.float16)
            xw = NB * D  # per-batch width

            # PE warmup: ~5us of dummy matmuls on a scratch tile so the HAM
            # clock gate reaches 8/8 before the first real matmul.
            w_sb = wpool.tile([128, 64], mybir.dt.float16)
            nc.vector.memset(w_sb[:], 0.0)
            w_ps = wpspool.tile([64, 64], mybir.dt.float32)
            for _ in range(NWARM):
                nc.tensor.matmul(w_ps[:], w_sb[:], w_sb[:], start=True, stop=True)

            def cast_x(b, q):
                qw = xw // 4
                lo = b * xw + q * qw
                nc.vector.tensor_copy(x_h[:, lo : lo + qw], x_sb[:, lo : lo + qw])

            def load_x(b, q, eng=None):
                qw = xw // 4
                lo = b * xw + q * qw
                n0 = b * N + q * (N // 4)
                (eng or nc.scalar).dma_start(
                    x_sb[:, lo : lo + qw],
                    x_ap[n0 : n0 + N // 4].rearrange("(g p) d -> p g d", p=128),
                )

            # Batch-0 x is the ramp-critical load: split it across BOTH HWDGE
            # rings (q0/q2 on ACT; q1/q3 on sync INTERLEAVED with the first A
            # tiles so a0 lands first and the A pipeline never falls behind).
            # Batch 1 follows on the ACT ring. Batches 2-3 are deferred into
            # the loop below: their deadlines are ~70/95us and issuing them
            # early would steal HBM bandwidth from A and the out stream
            # (backlog -> tail time).
            def a_dma(b, im):
                a_t = apool.tile(
                    [128, AMERGE * NB * 128], mybir.dt.float8e4, name="a_t"
                )
                nc.sync.dma_start(a_t[:], a_ap[b, im])
                return a_t

            load_x(0, 0)
            pre_a = {0: a_dma(0, 0)}
            load_x(0, 1, nc.sync)
            load_x(0, 3, nc.sync)
            pre_a[1] = a_dma(0, 1)
            load_x(0, 2)
            for q in range(4):
                load_x(1, q)
            for q in range(4):
                cast_x(0, q)

            o_t = None
            for b in range(BPC):
                for im in range(NB // AMERGE):
                    # one DMA covering AMERGE dst-strips of A (fp8 bytes)
                    if b == 0 and im in pre_a:
                        a_t = pre_a.pop(im)
                    else:
                        a_t = a_dma(b, im)
                    if b < 2 and im >= 4:
                        load_x(b + 2, im - 4)  # deferred x stream (batches 2-3)
                    if b + 1 < BPC and im >= 4:
                        # cast next batch's x in the LATE half of this batch:
                        # queued any earlier it head-of-line-blocks the PSUM
                        # drains behind it in the DVE FIFO while waiting for
                        # its x quarter, which stalls the whole out stream
                        cast_x(b + 1, im - 4)

                    def finish_strip(i, pt):
                        nonlocal o_t
                        g = b * NB + i
                        # last batch: per-strip out DMAs so the tail flush is
                        # one 256KB write instead of a 512KB straggler
                        om = 1 if b == BPC - 1 else OMERGE
                        if i % om == 0:
                            o_t = opool.tile([128, OMERGE * 2 * D], mybir.dt.float32)
                        oo = i % om
                        # row content: [scatter 256 | echo 256]
                        nc.vector.tensor_copy(o_t[:, oo * 2 * D : oo * 2 * D + D], pt[:])
                        nc.scalar.copy(
                            o_t[:, oo * 2 * D + D : (oo + 1) * 2 * D],
                            x_sb[:, g * D : (g + 1) * D],
                        )
                        if i % om == om - 1:
                            g0 = g - (om - 1)
                            nc.gpsimd.dma_start(
                                out_ap[g0 * 128 : (g0 + om) * 128, :].rearrange(
                                    "(gg p) c -> p gg c", p=128
                                ),
                                o_t[:, : om * 2 * D],
                            )

                    def mm(i, pt, asrc, j):
                        nc.tensor.matmul(
                            pt[:],
                            asrc[:, j * 128 : (j + 1) * 128],
                            x_h[:, (b * NB + j) * D : (b * NB + j + 1) * D],
                            start=(j == 0),
                            stop=(j == NB - 1),
                        )

                    if b == 0 and im == 0:
                        # ramp: batch-0 x arrives quarter by quarter; run the
                        # j-loop in quarter order across BOTH strips so the PE
                        # computes behind each arriving quarter instead of
                        # stalling strip 0 on the full batch.
                        pts = [
                            pspool.tile([128, D], mybir.dt.float32, name="pt")
                            for _ in range(AMERGE)
                        ]
                        for q in range(4):
                            for ii in range(AMERGE):
                                asrc = a_t[:, ii * NB * 128 : (ii + 1) * NB * 128]
                                for j in range(4 * q, 4 * q + 4):
                                    mm(ii, pts[ii], asrc, j)
                        for ii in range(AMERGE):
                            finish_strip(ii, pts[ii])
                    else:
                        for ii in range(AMERGE):
                            i = im * AMERGE + ii
                            asrc = a_t[:, ii * NB * 128 : (ii + 1) * NB * 128]
                            pt = pspool.tile([128, D], mybir.dt.float32)
                            for j in range(NB):
                                mm(i, pt, asrc, j)
                            finish_strip(i, pt)

    nc.compile()
    return nc


def _host_build_adjacency(batch_idx, src_idx, dst_idx):
    """Per-batch symmetric count matrices, laid out as lhsT blocks, fp8e4-encoded.

    Returns fp8e4m3 array [B, NB//AMERGE, 128, AMERGE, NB, 128]: a[b, im, s, ii, j, d]
    = M_b[j*128+s, (im*AMERGE+ii)*128+d] (M symmetric: [src, dst] block feeding
    dst-block im*AMERGE+ii from src-block j), im-group contiguous per s for DMA.
    """
    import ml_dtypes

    a = np.empty((B, NB // AMERGE, 128, AMERGE, NB, 128), dtype=np.uint8)
    order = np.argsort(batch_idx, kind="stable")
    bcounts = np.bincount(batch_idx.astype(np.int64), minlength=B)
    offs = np.zeros(B + 1, dtype=np.int64)
    np.cumsum(bcounts, out=offs[1:])
    src_s = src_idx[order].astype(np.int64)
    dst_s = dst_idx[order].astype(np.int64)
    for b in range(B):
        s = src_s[offs[b] : offs[b + 1]]
        d = dst_s[offs[b] : offs[b + 1]]
        ids = np.concatenate([d * N + s, s * N + d])
        m = np.bincount(ids, minlength=N * N)
        # counts <= 16 are exact in fp8e4m3; 17..255 round to nearest even
        # representable (max err ~3%, affects a handful of cells -> negligible)
        assert m.max() < 256, "count overflows fp8 LUT"
        # m[row, col]: row = src (lhsT partition), col = dst (M symmetric)
        mr = m.reshape(NB, 128, NB, 128)  # [J, s, I, d]
        isd = mr.transpose(2, 1, 0, 3).astype(np.uint8)  # [I, s, J, d]
        a[b] = isd.reshape(NB // AMERGE, AMERGE, 128, NB, 128).transpose(0, 2, 1, 3, 4)
    # integer counts -> fp8e4m3 bit patterns; exact for 0..16, even 18..32;
    # odd 17..31 round to nearest even representable (negligible: few cells)
    lut = np.arange(256).astype(ml_dtypes.float8_e4m3).view(np.uint8)
    return lut[a].view(ml_dtypes.float8_e4m3)


def kernel(x, batch_idx, src_idx, dst_idx):
    global _compiled
    from concourse import bass_utils

    assert x.shape == (B, N, D), x.shape
    a_all = _host_build_adjacency(batch_idx, src_idx, dst_idx)

    if _compiled is None:
        _compiled = _build_bass()
    nc = _compiled

    in_maps = []
    for c in range(NC):
        xs = np.ascontiguousarray(
            x[c * BPC : (c + 1) * BPC].reshape(BPC * N, D).astype(np.float32)
        )
        asrd = np.ascontiguousarray(a_all[c * BPC : (c + 1) * BPC])
        in_maps.append({"x": xs, "a": asrd})

    res = bass_utils.run_bass_kernel_spmd(nc, in_maps, core_ids=list(range(NC)))

    out = np.empty((B, N, 2 * D), dtype=np.float32)
    for c in range(NC):
        out[c * BPC : (c + 1) * BPC] = res.results[c]["out"].reshape(BPC, N, 2 * D)
    return out


# revision 22
# speedup vs baseline: 1.0152x; 1.0152x over previous
"""Trainium2 Bass kernel: bidirectional GNN message passing (scatter-add) + concat.

Computation (per batch b):
    out[b, :, 0:256]   = M_b @ x[b]        where M_b[i, j] = (# edges i<-j) + (# edges j<-i)
    out[b, :, 256:512] = x[b]

M_b is a symmetric count matrix built on the host from the edge indices (pure
index preprocessing; all x-dependent arithmetic runs on the NeuronCores).
Sharding: data-parallel over the batch dim, 4 batches per core on 8 cores.
On-device the scatter-add is computed as dense 128x128-block matmuls on the
tensor engine. The counts are stored in HBM directly as fp8e4m3 bytes (exact
for counts <= 16) so the PE consumes them with zero on-chip casting; x is the
f16 moving operand. Output rows are assembled as [scatter | x] in SBUF so the
store is one stream of 2KB-contiguous descriptors.
"""

import numpy as np

B, N, D = 32, 2048, 256
NC = 8                  # cores
BPC = B // NC           # batches per core = 4
NB = N // 128           # node blocks per batch = 16
G = BPC * NB            # node blocks per core = 64
AMERGE = 2              # dst strips per A DMA
OMERGE = 2              # strips per out DMA
NWARM = 32              # PE warmup matmuls (clock-gate ramp during initial DMA)

_compiled = None


def _build_bass():
    from contextlib import ExitStack
    import concourse.bass as bass
    import concourse.tile as tile
    from concourse import bacc, mybir

    nc = bacc.Bacc("TRN2", target_bir_lowering=False, debug=False, num_devices=NC)
    x_ap = nc.dram_tensor("x", [BPC * N, D], mybir.dt.float32, kind="ExternalInput").ap()
    # A layout [b, im, s, ii, J, d] fp8e4: each im-group of AMERGE dst-strips is a
    # flat [128, AMERGE*NB*128] block -> 4KB-contiguous DMA descriptor runs.
    a_ap = nc.dram_tensor(
        "a", [BPC, NB // AMERGE, 128, AMERGE * NB * 128], mybir.dt.float8e4, kind="ExternalInput"
    ).ap()
    out_ap = nc.dram_tensor("out", [BPC * N, 2 * D], mybir.dt.float32, kind="ExternalOutput").ap()

    with tile.TileContext(nc) as tc:
        with ExitStack() as ctx:
            xpool = ctx.enter_context(tc.tile_pool(name="x", bufs=1))
            xhpool = ctx.enter_context(tc.tile_pool(name="xh", bufs=1))
            # A prefetch capped at 4 strips: ~14us of PE runway while leaving
            # early HBM bandwidth to the batch-0 x load (ramp). Measured: 6
            # bufs steals ramp bandwidth from x and nets out slower.
            apool = ctx.enter_context(tc.tile_pool(name="a8", bufs=4))
            pspool = ctx.enter_context(tc.tile_pool(name="ps", bufs=6, space="PSUM"))
            wpool = ctx.enter_context(tc.tile_pool(name="wm", bufs=1))
            wpspool = ctx.enter_context(tc.tile_pool(name="wps", bufs=1, space="PSUM"))
            opool = ctx.enter_context(tc.tile_pool(name="o", bufs=6))

            # x resident in SBUF: [p, (g, d)] where node n = g*128 + p.
            x_sb = xpool.tile([128, G * D], mybir.dt.float32)
            x_h = xhpool.tile([128, G * D], mybir.dt.float16)
            xw = NB * D  # per-batch width

            # PE warmup: ~5us of dummy matmuls on a scratch tile so the HAM
            # clock gate reaches 8/8 before the first real matmul.
            w_sb = wpool.tile([128, 64], mybir.dt.float16)
            nc.vector.memset(w_sb[:], 0.0)
            w_ps = wpspool.tile([64, 64], mybir.dt.float32)
            for _ in range(NWARM):
                nc.tensor.matmul(w_ps[:], w_sb[:], w_sb[:], start=True, stop=True)

            def cast_x(b, q):
                qw = xw // 4
                lo = b * xw + q * qw
                nc.vector.tensor_copy(x_h[:, lo : lo + qw], x_sb[:, lo : lo + qw])

            def load_x(b, q, eng=None):
                qw = xw // 4
                lo = b * xw + q * qw
                n0 = b * N + q * (N // 4)
                (eng or nc.scalar).dma_start(
                    x_sb[:, lo : lo + qw],
                    x_ap[n0 : n0 + N // 4].rearrange("(g p) d -> p g d", p=128),
                )

            # Batch-0 x is the ramp-critical load: split it across BOTH HWDGE
            # rings (q0/q2 on ACT; q1/q3 on sync INTERLEAVED with the first A
            # tiles so a0 lands first and the A pipeline never falls behind).
            # Batch 1 follows on the ACT ring. Batches 2-3 are deferred into
            # the loop below: their deadlines are ~70/95us and issuing them
            # early would steal HBM bandwidth from A and the out stream
            # (backlog -> tail time).
            def a_dma(b, im):
                a_t = apool.tile(
                    [128, AMERGE * NB * 128], mybir.dt.float8e4, name="a_t"
                )
                nc.sync.dma_start(a_t[:], a_ap[b, im])
                return a_t

            load_x(0, 0)
            pre_a = {0: a_dma(0, 0)}
            load_x(0, 1, nc.sync)
            pre_a[1] = a_dma(0, 1)
            load_x(0, 3, nc.sync)
            load_x(0, 2)
            for q in range(4):
                load_x(1, q)
            for q in range(4):
                cast_x(0, q)

            o_t = None
            for b in range(BPC):
                for im in range(NB // AMERGE):
                    # one DMA covering AMERGE dst-strips of A (fp8 bytes)
                    if b == 0 and im in pre_a:
                        a_t = pre_a.pop(im)
                    else:
                        a_t = a_dma(b, im)
                    if b < 2 and im >= 4:
                        load_x(b + 2, im - 4)  # deferred x stream (batches 2-3)
                    if b + 1 < BPC and im >= 4:
                        # cast next batch's x in the LATE half of this batch:
                        # queued any earlier it head-of-line-blocks the PSUM
                        # drains behind it in the DVE FIFO while waiting for
                        # its x quarter, which stalls the whole out stream
                        cast_x(b + 1, im - 4)

                    def finish_strip(i, pt):
                        nonlocal o_t
                        g = b * NB + i
                        # last batch: per-strip out DMAs so the tail flush is
                        # one 256KB write instead of a 512KB straggler
                        om = 1 if b == BPC - 1 else OMERGE
                        if i % om == 0:
                            o_t = opool.tile([128, OMERGE * 2 * D], mybir.dt.float32)
                        oo = i % om
                        # row content: [scatter 256 | echo 256]
                        nc.vector.tensor_copy(o_t[:, oo * 2 * D : oo * 2 * D + D], pt[:])
                        nc.scalar.copy(
                            o_t[:, oo * 2 * D + D : (oo + 1) * 2 * D],
                            x_sb[:, g * D : (g + 1) * D],
                        )
                        if i % om == om - 1:
                            g0 = g - (om - 1)
                            nc.gpsimd.dma_start(
                                out_ap[g0 * 128 : (g0 + om) * 128, :].rearrange(
                                    "(gg p) c -> p gg c", p=128
                                ),
                                o_t[:, : om * 2 * D],
                            )

                    def mm(i, pt, asrc, j):
                        nc.tensor.matmul(
                            pt[:],
                            asrc[:, j * 128 : (j + 1) * 128],
                            x_h[:, (b * NB + j) * D : (b * NB + j + 1) * D],
                            start=(j == 0),
                            stop=(j == NB - 1),
                        )

                    if b == 0 and im == 0:
                        # ramp: batch-0 x arrives quarter by quarter; run the
                        # j-loop in quarter order across BOTH strips so the PE
                        # computes behind each arriving quarter instead of
                        # stalling strip 0 on the full batch.
                        pts = [
                            pspool.tile([128, D], mybir.dt.float32, name="pt")
                            for _ in range(AMERGE)
                        ]
                        for q in range(4):
                            for ii in range(AMERGE):
                                asrc = a_t[:, ii * NB * 128 : (ii + 1) * NB * 128]
                                for j in range(4 * q, 4 * q + 4):
                                    mm(ii, pts[ii], asrc, j)
                        for ii in range(AMERGE):
                            finish_strip(ii, pts[ii])
                    else:
                        for ii in range(AMERGE):
                            i = im * AMERGE + ii
                            asrc = a_t[:, ii * NB * 128 : (ii + 1) * NB * 128]
                            pt = pspool.tile([128, D], mybir.dt.float32)
                            for j in range(NB):
                                mm(i, pt, asrc, j)
                            finish_strip(i, pt)

    nc.compile()
    return nc


def _host_build_adjacency(batch_idx, src_idx, dst_idx):
    """Per-batch symmetric count matrices, laid out as lhsT blocks, fp8e4-encoded.

    Returns fp8e4m3 array [B, NB//AMERGE, 128, AMERGE, NB, 128]: a[b, im, s, ii, j, d]
    = M_b[j*128+s, (im*AMERGE+ii)*128+d] (M symmetric: [src, dst] block feeding
    dst-block im*AMERGE+ii from src-block j), im-group contiguous per s for DMA.
    """
    import ml_dtypes

    a = np.empty((B, NB // AMERGE, 128, AMERGE, NB, 128), dtype=np.uint8)
    order = np.argsort(batch_idx, kind="stable")
    bcounts = np.bincount(batch_idx.astype(np.int64), minlength=B)
    offs = np.zeros(B + 1, dtype=np.int64)
    np.cumsum(bcounts, out=offs[1:])
    src_s = src_idx[order].astype(np.int64)
    dst_s = dst_idx[order].astype(np.int64)
    for b in range(B):
        s = src_s[offs[b] : offs[b + 1]]
        d = dst_s[offs[b] : offs[b + 1]]
        ids = np.concatenate([d * N + s, s * N + d])
        m = np.bincount(ids, minlength=N * N)
        # counts <= 16 are exact in fp8e4m3; 17..255 round to nearest even
        # representable (max err ~3%, affects a handful of cells -> negligible)
        assert m.max() < 256, "count overflows fp8 LUT"
        # m[row, col]: row = src (lhsT partition), col = dst (M symmetric)
        mr = m.reshape(NB, 128, NB, 128)  # [J, s, I, d]
        isd = mr.transpose(2, 1, 0, 3).astype(np.uint8)  # [I, s, J, d]
        a[b] = isd.reshape(NB // AMERGE, AMERGE, 128, NB, 128).transpose(0, 2, 1, 3, 4)
    # integer counts -> fp8e4m3 bit patterns; exact for 0..16, even 18..32;
    # odd 17..31 round to nearest even representable (negligible: few cells)
    lut = np.arange(256).astype(ml_dtypes.float8_e4m3).view(np.uint8)
    return lut[a].view(ml_dtypes.float8_e4m3)


def kernel(x, batch_idx, src_idx, dst_idx):
    global _compiled
    from concourse import bass_utils

    assert x.shape == (B, N, D), x.shape
    a_all = _host_build_adjacency(batch_idx, src_idx, dst_idx)

    if _compiled is None:
        _compiled = _build_bass()
    nc = _compiled

    in_maps = []
    for c in range(NC):
        xs = np.ascontiguousarray(
            x[c * BPC : (c + 1) * BPC].reshape(BPC * N, D).astype(np.float32)
        )
        asrd = np.ascontiguousarray(a_all[c * BPC : (c + 1) * BPC])
        in_maps.append({"x": xs, "a": asrd})

    res = bass_utils.run_bass_kernel_spmd(nc, in_maps, core_ids=list(range(NC)))

    out = np.empty((B, N, 2 * D), dtype=np.float32)
    for c in range(NC):
        out[c * BPC : (c + 1) * BPC] = res.results[c]["out"].reshape(BPC, N, 2 * D)
    return out
